# revision 19
# baseline (speedup 1.0000x reference)
"""Trainium2 Bass kernel for nn_CrossLayer (B=256, F=39, E=16, I=741, C=1).

out[b, 0, i, e1, e2] = BN(cross)[b,i,e1,e2] * W[0,e1,e2]
  cross[b,i,e1,e2] = xj[b, rows[i], e1] * xi[b, cols[i], e2]
  BN over channel i with training-mode batch stats across (b, e1, e2).

Sharding: channels I=741 split across 8 cores (93 per core, zero-padded).
Each core sees the full batch for its channels, so BN stats are fully
local (no collectives).

BN stats are computed analytically from the COMPACT per-feature sums:
  mean[r,c]  = (1/(B*E^2)) * sum_b sJ[b,r]*sI[b,c]   (39x39 PE matmul)
  E[x^2][r,c]= (1/(B*E^2)) * sum_b qJ[b,r]*qI[b,c]
then per-channel scale/bias are gathered from the 39x39 maps with a
one-hot PE matmul (rows) + masked free-reduce (cols).  The stats
operands ride in a single packed "blob" DMA (~1.6 MB) so the chain can
finish while the big gathered operands are still streaming in.

Per-core device pipeline:
  DVE: t1 = A (x) X              outer product, b on partitions
                                 (stride-0 broadcast APs)
  PE : transpose 128x128 tiles   t1 -> PSUM, (e1,e2) now on partitions
  ACT/DVE: out = t1_T*VM + VA    per-partition scale/bias fuses the BN
                                 affine AND the W multiply in one op:
                                 VM[p,j] = s_j*W[p], VA[p,j] = b_j*W[p]
                                 (split ~4:1 between ScalarE and VectorE)
  DMA: out tiles -> HBM
"""

import numpy as np

import concourse.bacc as bacc
import concourse.mybir as mybir
from concourse.tile import TileContext
from concourse.bass_utils import run_bass_kernel_spmd
from bass_rust import add_dep_helper as _add_dep

B, F, E = 256, 39, 16
EE = E * E
I = 741  # strict lower triangle of (39, 39)
P = 128
NB = B // P  # 2 batch chunks on partitions
NE = EE // P  # 2 (e1,e2) chunks on partitions after transpose
NCORES = 8
JPC = 93  # padded channels per core (8*93 = 744 >= 741)
GJ = 8  # channels per pipeline group
NLD = 6  # input load interleave chunks
EPS = 1e-5
FCE = F * E  # 624

# blob column offsets (fp32 elements; blob is (128, BLOBZ))
_Z_XJC = 0                    # (P, NB*FCE)
_Z_XIC = _Z_XJC + NB * FCE    # (P, NB*FCE)
_Z_ID = _Z_XIC + NB * FCE     # (P, P) identity
_Z_G2 = _Z_ID + P             # (F, F) gamma scattered
_Z_B2 = _Z_G2 + F             # (F, F) beta scattered
_Z_GR = _Z_B2 + F             # (F, JPC) one-hot row select
_Z_GCD = _Z_GR + JPC          # (JPC, F) one-hot col mask
_Z_W = _Z_GCD + F             # (1, EE) W flat
BLOBZ = _Z_W + EE

_ROWS, _COLS = np.tril_indices(F, k=-1)

# module-level knobs for the test harness
TRACE = False
LAST_RESULT = None

_cached_nc = None

_f32 = mybir.dt.float32
_mult = mybir.AluOpType.mult
_add = mybir.AluOpType.add


def _groups():
    # small lead-in groups keep DVE stalls short while the stats chain is
    # still running; small tail groups shrink the final drain
    g = [(0, 2), (2, 2), (4, 2), (6, 2), (8, 2), (10, 2), (12, 2), (14, 2)]
    j0 = 16
    while j0 < 88:
        g.append((j0, min(GJ, 88 - j0)))
        j0 += GJ
    g += [(88, 2), (90, 2), (92, 1)]
    return g


def _ld_chunks():
    g = []
    step = (JPC + NLD - 1) // NLD
    j0 = 0
    while j0 < JPC:
        g.append((j0, min(step, JPC - j0)))
        j0 += step
    return g


def _build_program():
    nc = bacc.Bacc("TRN2", target_bir_lowering=False, debug=False,
                   num_devices=NCORES)
    FJE = JPC * E  # 1488

    ag = nc.dram_tensor("ag", (P, NB, FJE), _f32, kind="ExternalInput")
    xg = nc.dram_tensor("xg", (P, NB, FJE), _f32, kind="ExternalInput")
    blob = nc.dram_tensor("blob", (P, BLOBZ), _f32, kind="ExternalInput")
    # out[j, p, c2, c, b128] = result[b=c*128+b128, i=j, e=c2*128+p]
    out_t = nc.dram_tensor("out", (JPC, P, NE * NB * P), _f32,
                           kind="ExternalOutput")

    Ident = mybir.ActivationFunctionType.Identity

    with TileContext(nc) as tc:
        with tc.tile_pool(name="const", bufs=1) as cpool, \
             tc.tile_pool(name="work", bufs=3) as wpool:
            ppool_cm = tc.tile_pool(name="psum_st", bufs=1, space="PSUM")
            ppool = ppool_cm.__enter__()

            # --- one packed DMA for everything the stats path needs ---
            blob_sb = cpool.tile([P, BLOBZ], _f32)
            nc.sync.dma_start(blob_sb[:, :_Z_ID], blob[:, :_Z_ID])
            nc.sync.dma_start(blob_sb[:, _Z_ID:], blob[:, _Z_ID:])

            bv = blob_sb[:]
            xjc_sb = bv[:, _Z_XJC:_Z_XJC + NB * FCE]
            xic_sb = bv[:, _Z_XIC:_Z_XIC + NB * FCE]
            id_sb = bv[:, _Z_ID:_Z_ID + P]
            g2_sb = bv[:F, _Z_G2:_Z_G2 + F]
            b2_sb = bv[:F, _Z_B2:_Z_B2 + F]
            gr_sb = bv[:F, _Z_GR:_Z_GR + JPC]
            gcd_sb = bv[:JPC, _Z_GCD:_Z_GCD + F]
            w_sb = bv[:1, _Z_W:_Z_W + EE]

            # --- big gathered loads, interleaved by channel range ---
            a_sb = cpool.tile([P, NB * FJE], _f32)
            x_sb = cpool.tile([P, NB * FJE], _f32)
            a3 = a_sb[:].rearrange("p (c j e) -> p c j e", c=NB, j=JPC)
            x3 = x_sb[:].rearrange("p (c j e) -> p c j e", c=NB, j=JPC)
            ag3 = ag[:].rearrange("p c (j e) -> p c j e", j=JPC)
            xg3 = xg[:].rearrange("p c (j e) -> p c j e", j=JPC)
            for (r0, rn) in _ld_chunks():
                nc.sync.dma_start(a3[:, :, r0:r0 + rn, :],
                                  ag3[:, :, r0:r0 + rn, :])
                nc.sync.dma_start(x3[:, :, r0:r0 + rn, :],
                                  xg3[:, :, r0:r0 + rn, :])

            # ---- stats from compact tensors ----
            sAll = cpool.tile([P, 2 * NB * F], _f32)
            qAll = cpool.tile([P, 2 * NB * F], _f32)
            both = bv[:, _Z_XJC:_Z_XJC + 2 * NB * FCE]
            nc.vector.tensor_reduce(
                sAll[:].rearrange("p (t f) -> p t f", t=2 * NB),
                both.rearrange("p (t f e) -> p t f e", t=2 * NB, f=F),
                mybir.AxisListType.X, _add)
            sJ = sAll[:, :NB * F]
            sI = sAll[:, NB * F:]
            qJ = qAll[:, :NB * F]
            qI = qAll[:, NB * F:]

            # M1[r,c] = sum_b sJ[b,r]*sI[b,c]
            m1_ps = ppool.tile([F, F], _f32, tag="m1")
            for c in range(NB):
                nc.tensor.matmul(m1_ps[:], sJ[:, c * F:(c + 1) * F],
                                 sI[:, c * F:(c + 1) * F],
                                 start=(c == 0), stop=(c == NB - 1))

            sqAll = cpool.tile([P, 2 * NB * FCE], _f32)
            nc.scalar.square(sqAll[:], both)
            nc.vector.tensor_reduce(
                qAll[:].rearrange("p (t f) -> p t f", t=2 * NB),
                sqAll[:].rearrange("p (t f e) -> p t f e", t=2 * NB, f=F),
                mybir.AxisListType.X, _add)
            m2_ps = ppool.tile([F, F], _f32, tag="m2")
            for c in range(NB):
                nc.tensor.matmul(m2_ps[:], qJ[:, c * F:(c + 1) * F],
                                 qI[:, c * F:(c + 1) * F],
                                 start=(c == 0), stop=(c == NB - 1))

            inv_n = 1.0 / float(B * EE)
            mean2 = cpool.tile([F, F], _f32)
            nc.vector.tensor_scalar_mul(mean2[:], m1_ps[:], inv_n)

            m2sq = cpool.tile([F, F], _f32)
            varp = cpool.tile([F, F], _f32)
            nc.vector.tensor_mul(m2sq[:], mean2[:], mean2[:])
            # varp = M2*inv_n - mean2^2, then + eps (fused stt + ts)
            nc.vector.scalar_tensor_tensor(
                varp[:], m2_ps[:], inv_n, m2sq[:], _mult,
                mybir.AluOpType.subtract)
            nc.vector.tensor_scalar_add(varp[:], varp[:], EPS)

            # rstd = rsqrt(var+eps): reciprocal -> sqrt -> one Newton step
            inv = cpool.tile([F, F], _f32)
            r0t = cpool.tile([F, F], _f32)
            nc.vector.reciprocal(inv[:], varp[:])
            nc.scalar.sqrt(r0t[:], inv[:])
            r0sq = cpool.tile([F, F], _f32)
            ut = cpool.tile([F, F], _f32)
            rstd = cpool.tile([F, F], _f32)
            nc.vector.tensor_mul(r0sq[:], r0t[:], r0t[:])
            nc.vector.tensor_mul(ut[:], varp[:], r0sq[:])
            nc.vector.tensor_scalar(ut[:], ut[:], -0.5, 1.5, _mult, _add)
            nc.vector.tensor_mul(rstd[:], r0t[:], ut[:])

            sc2 = cpool.tile([F, F], _f32)
            bi2 = cpool.tile([F, F], _f32)
            tmp2 = cpool.tile([F, F], _f32)
            nc.vector.tensor_mul(sc2[:], g2_sb, rstd[:])
            nc.vector.tensor_mul(tmp2[:], mean2[:], sc2[:])
            nc.vector.tensor_sub(bi2[:], b2_sb, tmp2[:])

            # gather per-channel values: v[j] = V[rows[j], cols[j]]
            # rows via one-hot matmul, cols via masked free-reduce,
            # then transpose the (93,1) column to a (1,93) row via PE.
            def gather_row(v2d, name):
                gps = ppool.tile([JPC, F], _f32, tag=f"misc_{name}")
                nc.tensor.matmul(gps[:], gr_sb, v2d[:])
                o1 = cpool.tile([JPC, F], _f32, tag=f"o1_{name}")
                nc.vector.tensor_copy(o1[:], gps[:])
                junk = cpool.tile([JPC, F], _f32, tag=f"junk_{name}")
                vcol = cpool.tile([JPC, 1], _f32, tag=f"vcol_{name}")
                nc.vector.tensor_mul(junk[:], o1[:], gcd_sb)
                nc.vector.tensor_reduce(vcol[:], junk[:],
                                        mybir.AxisListType.X, _add)
                vrow_ps = ppool.tile([1, JPC], _f32, tag=f"misc_{name}")
                nc.tensor.matmul(vrow_ps[:], vcol[:], id_sb[:JPC, :JPC])
                vrow = cpool.tile([1, JPC], _f32, tag=f"vr_{name}")
                nc.vector.tensor_copy(vrow[:], vrow_ps[:])
                return vrow

            scv = gather_row(sc2, "s")
            biv = gather_row(bi2, "b")

            # VM[p, c2, j] = W[c2*128+p] * s_j ; VA likewise with b_j
            # both packed into one PSUM bank, evacuated with one copy
            vv = cpool.tile([P, 2 * NE * JPC], _f32)
            vv_ps = ppool.tile([P, 2 * NE * JPC], _f32, tag="vvps")
            for c2 in range(NE):
                wt = w_sb[:, c2 * P:(c2 + 1) * P]
                nc.tensor.matmul(vv_ps[:, c2 * JPC:(c2 + 1) * JPC], wt, scv[:],
                                 start=True, stop=True)
                nc.tensor.matmul(
                    vv_ps[:, (NE + c2) * JPC:(NE + c2 + 1) * JPC], wt, biv[:],
                    start=True, stop=True)
            vv_inst = nc.vector.tensor_copy(vv[:], vv_ps[:])
            vm = vv[:, :NE * JPC]
            va = vv[:, NE * JPC:]

            # release the stats PSUM banks; the transpose pool gets all 8
            ppool_cm.__exit__(None, None, None)
            trpool_cm = tc.tile_pool(name="psum_tr", bufs=8, space="PSUM")
            trpool = trpool_cm.__enter__()

            # ---- main pipeline over channel groups of GJ ----
            n_aff = 0
            for (g0, gn) in _groups():
                t1 = wpool.tile([P, NB * GJ * EE], _f32, tag="t1")
                t1v = t1[:].rearrange("p (c j f) -> p c j f", c=NB, j=GJ)

                for c in range(NB):
                    a_ap = a3[:, c, g0:g0 + gn, :].unsqueeze(3) \
                        .broadcast_to((P, gn, E, E))
                    x_ap = x3[:, c, g0:g0 + gn, :].unsqueeze(2) \
                        .broadcast_to((P, gn, E, E))
                    o_ap = t1v[:, c, :gn, :].rearrange(
                        "p j (e1 e2) -> p j e1 e2", e1=E)
                    t1_inst = nc.vector.tensor_tensor(o_ap, a_ap, x_ap, _mult)
                    if gn > 2:
                        # keep the DVE queue clear for the tiny stats-chain
                        # ops: order the bulk outer-product ops after the
                        # chain's last DVE op (same engine -> pure ordering)
                        _add_dep(t1_inst.ins, vv_inst.ins, sync=False,
                                 reason="stats chain before bulk outer products")

                og = wpool.tile([P, GJ * NE * NB * P], _f32, tag="og")
                og4 = og[:].rearrange("p (j c2 f) -> p j c2 f", j=GJ, c2=NE)

                for jj in range(gn):
                    j = g0 + jj
                    # one PSUM bank per channel: 4 transposed 128x128 tiles
                    tr = trpool.tile([P, NE * NB * P], _f32, tag="tr")
                    tr4 = tr[:].rearrange("p (c2 c b) -> p c2 c b", c2=NE, c=NB)
                    for c2 in range(NE):
                        for c in range(NB):
                            nc.tensor.transpose(
                                tr4[:, c2, c, :],
                                t1v[:, c, jj, c2 * P:(c2 + 1) * P],
                                id_sb)
                    for c2 in range(NE):
                        n_aff += 1
                        if n_aff <= 6 or n_aff % 5 == 0:
                            # ~1/5 of the affine ops on VectorE
                            nc.vector.tensor_scalar(
                                og4[:, jj, c2, :], tr4[:, c2, :, :],
                                vm[:, c2 * JPC + j:c2 * JPC + j + 1],
                                va[:, c2 * JPC + j:c2 * JPC + j + 1],
                                _mult, _add)
                        else:
                            nc.scalar.activation(
                                og4[:, jj, c2, :], tr4[:, c2, :, :], Ident,
                                bias=va[:, c2 * JPC + j:c2 * JPC + j + 1],
                                scale=vm[:, c2 * JPC + j:c2 * JPC + j + 1])

                nc.sync.dma_start(
                    out_t[g0:g0 + gn, :, :].rearrange("j p f -> p j f"),
                    og4[:, :gn, :, :].rearrange("p j c2 f -> p j (c2 f)"))

            trpool_cm.__exit__(None, None, None)

    nc.compile()
    return nc


def _shard_inputs(xi, xj, W, gamma, beta):
    """Host-side gather: per-core per-channel A (= xj rows) / X (= xi cols),
    plus a packed blob of compact tensors and static one-hot index maps."""
    FJE = JPC * E
    # shared pieces
    xjc = xj.reshape(NB, P, FCE).transpose(1, 0, 2).reshape(P, NB * FCE)
    xic = xi.reshape(NB, P, FCE).transpose(1, 0, 2).reshape(P, NB * FCE)
    g2d = np.ones((F, F), dtype=np.float32)
    b2d = np.zeros((F, F), dtype=np.float32)
    g2d[_ROWS, _COLS] = gamma
    b2d[_ROWS, _COLS] = beta

    in_maps = []
    for k in range(NCORES):
        j0 = k * JPC
        j1 = min(j0 + JPC, I)
        nj = j1 - j0
        rows = np.ones(JPC, dtype=np.int64)  # pad -> (1, 0)
        cols = np.zeros(JPC, dtype=np.int64)
        rows[:nj] = _ROWS[j0:j1]
        cols[:nj] = _COLS[j0:j1]

        a_k = np.zeros((P, NB, FJE), dtype=np.float32)
        x_k = np.zeros((P, NB, FJE), dtype=np.float32)
        a_full = xj[:, rows[:nj], :].reshape(NB, P, nj * E)
        x_full = xi[:, cols[:nj], :].reshape(NB, P, nj * E)
        a_k[:, :, :nj * E] = a_full.transpose(1, 0, 2)
        x_k[:, :, :nj * E] = x_full.transpose(1, 0, 2)

        bl = np.zeros((P, BLOBZ), dtype=np.float32)
        bl[:, _Z_XJC:_Z_XJC + NB * FCE] = xjc
        bl[:, _Z_XIC:_Z_XIC + NB * FCE] = xic
        bl[:, _Z_ID:_Z_ID + P] = np.eye(P, dtype=np.float32)
        bl[:F, _Z_G2:_Z_G2 + F] = g2d
        bl[:F, _Z_B2:_Z_B2 + F] = b2d
        bl[rows, _Z_GR + np.arange(JPC)] = 1.0
        bl[np.arange(JPC), _Z_GCD + cols] = 1.0
        bl[0, _Z_W:_Z_W + EE] = W.reshape(EE)

        in_maps.append({"ag": a_k, "xg": x_k, "blob": bl})
    return in_maps


def kernel(xi, xj, W, gamma, beta):
    global _cached_nc, LAST_RESULT
    xi = np.ascontiguousarray(np.asarray(xi), dtype=np.float32)
    xj = np.ascontiguousarray(np.asarray(xj), dtype=np.float32)
    W = np.asarray(W, dtype=np.float32)
    gamma = np.asarray(gamma, dtype=np.float32)
    beta = np.asarray(beta, dtype=np.float32)

    if _cached_nc is None:
        _cached_nc = _build_program()
    nc = _cached_nc

    in_maps = _shard_inputs(xi, xj, W, gamma, beta)
    res = run_bass_kernel_spmd(nc, in_maps, core_ids=list(range(NCORES)),
                               trace=TRACE)
    LAST_RESULT = res

    full = np.empty((B, I, EE), dtype=np.float32)
    for k in range(NCORES):
        j0 = k * JPC
        j1 = min(j0 + JPC, I)
        nj = j1 - j0
        r = res.results[k]["out"].reshape(JPC, P, NE, NB, P)
        # r[j, p, c2, c, b128] = out[b=c*128+b128, i=j0+j, e=c2*128+p]
        full[:, j0:j1, :] = (
            r[:nj].transpose(3, 4, 0, 2, 1).reshape(B, nj, EE))
    return full.reshape(B, 1, I, E, E)


# revision 20
# speedup vs baseline: 1.0072x; 1.0072x over previous
"""Trainium2 Bass kernel for nn_CrossLayer (B=256, F=39, E=16, I=741, C=1).

out[b, 0, i, e1, e2] = BN(cross)[b,i,e1,e2] * W[0,e1,e2]
  cross[b,i,e1,e2] = xj[b, rows[i], e1] * xi[b, cols[i], e2]
  BN over channel i with training-mode batch stats across (b, e1, e2).

Sharding: channels I=741 split across 8 cores (93 per core, zero-padded).
Each core sees the full batch for its channels, so BN stats are fully
local (no collectives).

BN stats are computed analytically from the COMPACT per-feature sums:
  mean[r,c]  = (1/(B*E^2)) * sum_b sJ[b,r]*sI[b,c]   (39x39 PE matmul)
  E[x^2][r,c]= (1/(B*E^2)) * sum_b qJ[b,r]*qI[b,c]
then per-channel scale/bias are gathered from the 39x39 maps with a
one-hot PE matmul (rows) + masked free-reduce (cols).  The stats
operands ride in a single packed "blob" DMA (~1.6 MB) so the chain can
finish while the big gathered operands are still streaming in.

Per-core device pipeline:
  DVE: t1 = A (x) X              outer product, b on partitions
                                 (stride-0 broadcast APs)
  PE : transpose 128x128 tiles   t1 -> PSUM, (e1,e2) now on partitions
  ACT/DVE: out = t1_T*VM + VA    per-partition scale/bias fuses the BN
                                 affine AND the W multiply in one op:
                                 VM[p,j] = s_j*W[p], VA[p,j] = b_j*W[p]
                                 (split ~4:1 between ScalarE and VectorE)
  DMA: out tiles -> HBM
"""

import numpy as np

import concourse.bacc as bacc
import concourse.mybir as mybir
from concourse.tile import TileContext
from concourse.bass_utils import run_bass_kernel_spmd
from bass_rust import add_dep_helper as _add_dep

B, F, E = 256, 39, 16
EE = E * E
I = 741  # strict lower triangle of (39, 39)
P = 128
NB = B // P  # 2 batch chunks on partitions
NE = EE // P  # 2 (e1,e2) chunks on partitions after transpose
NCORES = 8
JPC = 93  # padded channels per core (8*93 = 744 >= 741)
GJ = 8  # channels per pipeline group
NLD = 6  # input load interleave chunks
EPS = 1e-5
FCE = F * E  # 624

# blob column offsets (fp32 elements; blob is (128, BLOBZ))
_Z_XJC = 0                    # (P, NB*FCE)
_Z_XIC = _Z_XJC + NB * FCE    # (P, NB*FCE)
_Z_ID = _Z_XIC + NB * FCE     # (P, P) identity
_Z_G2 = _Z_ID + P             # (F, F) gamma scattered
_Z_B2 = _Z_G2 + F             # (F, F) beta scattered
_Z_GR = _Z_B2 + F             # (F, JPC) one-hot row select
_Z_GCD = _Z_GR + JPC          # (JPC, F) one-hot col mask
_Z_W = _Z_GCD + F             # (1, EE) W flat
BLOBZ = _Z_W + EE

_ROWS, _COLS = np.tril_indices(F, k=-1)

# module-level knobs for the test harness
TRACE = False
LAST_RESULT = None

_cached_nc = None

_f32 = mybir.dt.float32
_mult = mybir.AluOpType.mult
_add = mybir.AluOpType.add


def _groups():
    # small lead-in groups keep DVE stalls short while the stats chain is
    # still running; small tail groups shrink the final drain
    g = [(0, 2), (2, 2), (4, 2), (6, 2), (8, 2), (10, 2), (12, 2), (14, 2)]
    j0 = 16
    while j0 < 88:
        g.append((j0, min(GJ, 88 - j0)))
        j0 += GJ
    g += [(88, 2), (90, 2), (92, 1)]
    return g


def _ld_chunks():
    g = []
    step = (JPC + NLD - 1) // NLD
    j0 = 0
    while j0 < JPC:
        g.append((j0, min(step, JPC - j0)))
        j0 += step
    return g


def _build_program():
    nc = bacc.Bacc("TRN2", target_bir_lowering=False, debug=False,
                   num_devices=NCORES)
    FJE = JPC * E  # 1488

    ag = nc.dram_tensor("ag", (P, NB, FJE), _f32, kind="ExternalInput")
    xg = nc.dram_tensor("xg", (P, NB, FJE), _f32, kind="ExternalInput")
    blob = nc.dram_tensor("blob", (P, BLOBZ), _f32, kind="ExternalInput")
    # out[j, p, c2, c, b128] = result[b=c*128+b128, i=j, e=c2*128+p]
    out_t = nc.dram_tensor("out", (JPC, P, NE * NB * P), _f32,
                           kind="ExternalOutput")

    Ident = mybir.ActivationFunctionType.Identity

    with TileContext(nc) as tc:
        with tc.tile_pool(name="const", bufs=1) as cpool, \
             tc.tile_pool(name="work", bufs=3) as wpool:
            ppool_cm = tc.tile_pool(name="psum_st", bufs=1, space="PSUM")
            ppool = ppool_cm.__enter__()

            # --- one packed DMA for everything the stats path needs ---
            blob_sb = cpool.tile([P, BLOBZ], _f32)
            nc.sync.dma_start(blob_sb[:, :_Z_ID], blob[:, :_Z_ID])
            nc.sync.dma_start(blob_sb[:, _Z_ID:], blob[:, _Z_ID:])

            bv = blob_sb[:]
            xjc_sb = bv[:, _Z_XJC:_Z_XJC + NB * FCE]
            xic_sb = bv[:, _Z_XIC:_Z_XIC + NB * FCE]
            id_sb = bv[:, _Z_ID:_Z_ID + P]
            g2_sb = bv[:F, _Z_G2:_Z_G2 + F]
            b2_sb = bv[:F, _Z_B2:_Z_B2 + F]
            gr_sb = bv[:F, _Z_GR:_Z_GR + JPC]
            gcd_sb = bv[:JPC, _Z_GCD:_Z_GCD + F]
            w_sb = bv[:1, _Z_W:_Z_W + EE]

            # --- big gathered loads, interleaved by channel range ---
            a_sb = cpool.tile([P, NB * FJE], _f32)
            x_sb = cpool.tile([P, NB * FJE], _f32)
            a3 = a_sb[:].rearrange("p (c j e) -> p c j e", c=NB, j=JPC)
            x3 = x_sb[:].rearrange("p (c j e) -> p c j e", c=NB, j=JPC)
            ag3 = ag[:].rearrange("p c (j e) -> p c j e", j=JPC)
            xg3 = xg[:].rearrange("p c (j e) -> p c j e", j=JPC)
            for (r0, rn) in _ld_chunks():
                nc.sync.dma_start(a3[:, :, r0:r0 + rn, :],
                                  ag3[:, :, r0:r0 + rn, :])
                nc.sync.dma_start(x3[:, :, r0:r0 + rn, :],
                                  xg3[:, :, r0:r0 + rn, :])

            # ---- stats from compact tensors ----
            sAll = cpool.tile([P, 2 * NB * F], _f32)
            qAll = cpool.tile([P, 2 * NB * F], _f32)
            both = bv[:, _Z_XJC:_Z_XJC + 2 * NB * FCE]
            nc.vector.tensor_reduce(
                sAll[:].rearrange("p (t f) -> p t f", t=2 * NB),
                both.rearrange("p (t f e) -> p t f e", t=2 * NB, f=F),
                mybir.AxisListType.X, _add)
            sJ = sAll[:, :NB * F]
            sI = sAll[:, NB * F:]
            qJ = qAll[:, :NB * F]
            qI = qAll[:, NB * F:]

            # M1[r,c] = sum_b sJ[b,r]*sI[b,c]
            m1_ps = ppool.tile([F, F], _f32, tag="m1")
            for c in range(NB):
                nc.tensor.matmul(m1_ps[:], sJ[:, c * F:(c + 1) * F],
                                 sI[:, c * F:(c + 1) * F],
                                 start=(c == 0), stop=(c == NB - 1))

            sqAll = cpool.tile([P, 2 * NB * FCE], _f32)
            nc.scalar.square(sqAll[:], both)
            nc.vector.tensor_reduce(
                qAll[:].rearrange("p (t f) -> p t f", t=2 * NB),
                sqAll[:].rearrange("p (t f e) -> p t f e", t=2 * NB, f=F),
                mybir.AxisListType.X, _add)
            m2_ps = ppool.tile([F, F], _f32, tag="m2")
            for c in range(NB):
                nc.tensor.matmul(m2_ps[:], qJ[:, c * F:(c + 1) * F],
                                 qI[:, c * F:(c + 1) * F],
                                 start=(c == 0), stop=(c == NB - 1))

            inv_n = 1.0 / float(B * EE)
            mean2 = cpool.tile([F, F], _f32)
            nc.vector.tensor_scalar_mul(mean2[:], m1_ps[:], inv_n)

            m2sq = cpool.tile([F, F], _f32)
            varp = cpool.tile([F, F], _f32)
            nc.vector.tensor_mul(m2sq[:], mean2[:], mean2[:])
            # varp = M2*inv_n - mean2^2, then + eps (fused stt + ts)
            nc.vector.scalar_tensor_tensor(
                varp[:], m2_ps[:], inv_n, m2sq[:], _mult,
                mybir.AluOpType.subtract)
            nc.vector.tensor_scalar_add(varp[:], varp[:], EPS)

            # rstd = rsqrt(var+eps): reciprocal -> sqrt -> one Newton step
            inv = cpool.tile([F, F], _f32)
            r0t = cpool.tile([F, F], _f32)
            nc.vector.reciprocal(inv[:], varp[:])
            nc.scalar.sqrt(r0t[:], inv[:])
            r0sq = cpool.tile([F, F], _f32)
            ut = cpool.tile([F, F], _f32)
            rstd = cpool.tile([F, F], _f32)
            nc.vector.tensor_mul(r0sq[:], r0t[:], r0t[:])
            nc.vector.tensor_mul(ut[:], varp[:], r0sq[:])
            nc.vector.tensor_scalar(ut[:], ut[:], -0.5, 1.5, _mult, _add)
            nc.vector.tensor_mul(rstd[:], r0t[:], ut[:])

            sc2 = cpool.tile([F, F], _f32)
            bi2 = cpool.tile([F, F], _f32)
            tmp2 = cpool.tile([F, F], _f32)
            nc.vector.tensor_mul(sc2[:], g2_sb, rstd[:])
            nc.vector.tensor_mul(tmp2[:], mean2[:], sc2[:])
            nc.vector.tensor_sub(bi2[:], b2_sb, tmp2[:])

            # gather per-channel values: v[j] = V[rows[j], cols[j]]
            # rows via one-hot matmul, cols via masked free-reduce,
            # then transpose the (93,1) column to a (1,93) row via PE.
            def gather_row(v2d, name):
                gps = ppool.tile([JPC, F], _f32, tag=f"misc_{name}")
                nc.tensor.matmul(gps[:], gr_sb, v2d[:])
                o1 = cpool.tile([JPC, F], _f32, tag=f"o1_{name}")
                nc.vector.tensor_copy(o1[:], gps[:])
                junk = cpool.tile([JPC, F], _f32, tag=f"junk_{name}")
                vcol = cpool.tile([JPC, 1], _f32, tag=f"vcol_{name}")
                nc.vector.tensor_mul(junk[:], o1[:], gcd_sb)
                nc.vector.tensor_reduce(vcol[:], junk[:],
                                        mybir.AxisListType.X, _add)
                vrow_ps = ppool.tile([1, JPC], _f32, tag=f"misc_{name}")
                nc.tensor.matmul(vrow_ps[:], vcol[:], id_sb[:JPC, :JPC])
                vrow = cpool.tile([1, JPC], _f32, tag=f"vr_{name}")
                nc.vector.tensor_copy(vrow[:], vrow_ps[:])
                return vrow

            scv = gather_row(sc2, "s")
            biv = gather_row(bi2, "b")

            # VM[p, c2, j] = W[c2*128+p] * s_j ; VA likewise with b_j
            # both packed into one PSUM bank, evacuated with one copy
            vv = cpool.tile([P, 2 * NE * JPC], _f32)
            vv_ps = ppool.tile([P, 2 * NE * JPC], _f32, tag="vvps")
            for c2 in range(NE):
                wt = w_sb[:, c2 * P:(c2 + 1) * P]
                nc.tensor.matmul(vv_ps[:, c2 * JPC:(c2 + 1) * JPC], wt, scv[:],
                                 start=True, stop=True)
                nc.tensor.matmul(
                    vv_ps[:, (NE + c2) * JPC:(NE + c2 + 1) * JPC], wt, biv[:],
                    start=True, stop=True)
            vv_inst = nc.vector.tensor_copy(vv[:], vv_ps[:])
            vm = vv[:, :NE * JPC]
            va = vv[:, NE * JPC:]

            # release the stats PSUM banks; the transpose pool gets all 8
            ppool_cm.__exit__(None, None, None)
            trpool_cm = tc.tile_pool(name="psum_tr", bufs=8, space="PSUM")
            trpool = trpool_cm.__enter__()

            # ---- main pipeline over channel groups of GJ ----
            n_aff = 0
            for (g0, gn) in _groups():
                t1 = wpool.tile([P, NB * GJ * EE], _f32, tag="t1")
                t1v = t1[:].rearrange("p (c j f) -> p c j f", c=NB, j=GJ)

                for c in range(NB):
                    a_ap = a3[:, c, g0:g0 + gn, :].unsqueeze(3) \
                        .broadcast_to((P, gn, E, E))
                    x_ap = x3[:, c, g0:g0 + gn, :].unsqueeze(2) \
                        .broadcast_to((P, gn, E, E))
                    o_ap = t1v[:, c, :gn, :].rearrange(
                        "p j (e1 e2) -> p j e1 e2", e1=E)
                    t1_inst = nc.vector.tensor_tensor(o_ap, a_ap, x_ap, _mult)
                    if gn > 2:
                        # keep the DVE queue clear for the tiny stats-chain
                        # ops: order the bulk outer-product ops after the
                        # chain's last DVE op (same engine -> pure ordering)
                        _add_dep(t1_inst.ins, vv_inst.ins, sync=False,
                                 reason="stats chain before bulk outer products")

                og = wpool.tile([P, GJ * NE * NB * P], _f32, tag="og")
                og4 = og[:].rearrange("p (j c2 f) -> p j c2 f", j=GJ, c2=NE)

                for jj in range(gn):
                    j = g0 + jj
                    # one PSUM bank per channel: 4 transposed 128x128 tiles
                    tr = trpool.tile([P, NE * NB * P], _f32, tag="tr")
                    tr4 = tr[:].rearrange("p (c2 c b) -> p c2 c b", c2=NE, c=NB)
                    for c2 in range(NE):
                        for c in range(NB):
                            nc.tensor.transpose(
                                tr4[:, c2, c, :],
                                t1v[:, c, jj, c2 * P:(c2 + 1) * P],
                                id_sb)
                    for c2 in range(NE):
                        n_aff += 1
                        if n_aff % 5 == 0:
                            # ~1/5 of the affine ops on VectorE
                            nc.vector.tensor_scalar(
                                og4[:, jj, c2, :], tr4[:, c2, :, :],
                                vm[:, c2 * JPC + j:c2 * JPC + j + 1],
                                va[:, c2 * JPC + j:c2 * JPC + j + 1],
                                _mult, _add)
                        else:
                            nc.scalar.activation(
                                og4[:, jj, c2, :], tr4[:, c2, :, :], Ident,
                                bias=va[:, c2 * JPC + j:c2 * JPC + j + 1],
                                scale=vm[:, c2 * JPC + j:c2 * JPC + j + 1])

                nc.sync.dma_start(
                    out_t[g0:g0 + gn, :, :].rearrange("j p f -> p j f"),
                    og4[:, :gn, :, :].rearrange("p j c2 f -> p j (c2 f)"))

            trpool_cm.__exit__(None, None, None)

    nc.compile()
    return nc


def _shard_inputs(xi, xj, W, gamma, beta):
    """Host-side gather: per-core per-channel A (= xj rows) / X (= xi cols),
    plus a packed blob of compact tensors and static one-hot index maps."""
    FJE = JPC * E
    # shared pieces
    xjc = xj.reshape(NB, P, FCE).transpose(1, 0, 2).reshape(P, NB * FCE)
    xic = xi.reshape(NB, P, FCE).transpose(1, 0, 2).reshape(P, NB * FCE)
    g2d = np.ones((F, F), dtype=np.float32)
    b2d = np.zeros((F, F), dtype=np.float32)
    g2d[_ROWS, _COLS] = gamma
    b2d[_ROWS, _COLS] = beta

    in_maps = []
    for k in range(NCORES):
        j0 = k * JPC
        j1 = min(j0 + JPC, I)
        nj = j1 - j0
        rows = np.ones(JPC, dtype=np.int64)  # pad -> (1, 0)
        cols = np.zeros(JPC, dtype=np.int64)
        rows[:nj] = _ROWS[j0:j1]
        cols[:nj] = _COLS[j0:j1]

        a_k = np.zeros((P, NB, FJE), dtype=np.float32)
        x_k = np.zeros((P, NB, FJE), dtype=np.float32)
        a_full = xj[:, rows[:nj], :].reshape(NB, P, nj * E)
        x_full = xi[:, cols[:nj], :].reshape(NB, P, nj * E)
        a_k[:, :, :nj * E] = a_full.transpose(1, 0, 2)
        x_k[:, :, :nj * E] = x_full.transpose(1, 0, 2)

        bl = np.zeros((P, BLOBZ), dtype=np.float32)
        bl[:, _Z_XJC:_Z_XJC + NB * FCE] = xjc
        bl[:, _Z_XIC:_Z_XIC + NB * FCE] = xic
        bl[:, _Z_ID:_Z_ID + P] = np.eye(P, dtype=np.float32)
        bl[:F, _Z_G2:_Z_G2 + F] = g2d
        bl[:F, _Z_B2:_Z_B2 + F] = b2d
        bl[rows, _Z_GR + np.arange(JPC)] = 1.0
        bl[np.arange(JPC), _Z_GCD + cols] = 1.0
        bl[0, _Z_W:_Z_W + EE] = W.reshape(EE)

        in_maps.append({"ag": a_k, "xg": x_k, "blob": bl})
    return in_maps


def kernel(xi, xj, W, gamma, beta):
    global _cached_nc, LAST_RESULT
    xi = np.ascontiguousarray(np.asarray(xi), dtype=np.float32)
    xj = np.ascontiguousarray(np.asarray(xj), dtype=np.float32)
    W = np.asarray(W, dtype=np.float32)
    gamma = np.asarray(gamma, dtype=np.float32)
    beta = np.asarray(beta, dtype=np.float32)

    if _cached_nc is None:
        _cached_nc = _build_program()
    nc = _cached_nc

    in_maps = _shard_inputs(xi, xj, W, gamma, beta)
    res = run_bass_kernel_spmd(nc, in_maps, core_ids=list(range(NCORES)),
                               trace=TRACE)
    LAST_RESULT = res

    full = np.empty((B, I, EE), dtype=np.float32)
    for k in range(NCORES):
        j0 = k * JPC
        j1 = min(j0 + JPC, I)
        nj = j1 - j0
        r = res.results[k]["out"].reshape(JPC, P, NE, NB, P)
        # r[j, p, c2, c, b128] = out[b=c*128+b128, i=j0+j, e=c2*128+p]
        full[:, j0:j1, :] = (
            r[:nj].transpose(3, 4, 0, 2, 1).reshape(B, nj, EE))
    return full.reshape(B, 1, I, E, E)


# revision 21
# speedup vs baseline: 1.0405x; 1.0330x over previous
"""Trainium2 Bass kernel for nn_CrossLayer (B=256, F=39, E=16, I=741, C=1).

out[b, 0, i, e1, e2] = BN(cross)[b,i,e1,e2] * W[0,e1,e2]
  cross[b,i,e1,e2] = xj[b, rows[i], e1] * xi[b, cols[i], e2]
  BN over channel i with training-mode batch stats across (b, e1, e2).

Sharding: channels I=741 split across 8 cores (93 per core, zero-padded).
Each core sees the full batch for its channels, so BN stats are fully
local (no collectives).

BN stats are computed analytically from the COMPACT per-feature sums:
  mean[r,c]  = (1/(B*E^2)) * sum_b sJ[b,r]*sI[b,c]   (39x39 PE matmul)
  E[x^2][r,c]= (1/(B*E^2)) * sum_b qJ[b,r]*qI[b,c]
then per-channel scale/bias are gathered from the 39x39 maps with a
one-hot PE matmul (rows) + masked free-reduce (cols).  The stats
operands ride in a single packed "blob" DMA (~1.6 MB) so the chain can
finish while the big gathered operands are still streaming in.

Per-core device pipeline:
  DVE: t1 = A (x) X              outer product, b on partitions
                                 (stride-0 broadcast APs)
  PE : transpose 128x128 tiles   t1 -> PSUM, (e1,e2) now on partitions
  ACT/DVE: out = t1_T*VM + VA    per-partition scale/bias fuses the BN
                                 affine AND the W multiply in one op:
                                 VM[p,j] = s_j*W[p], VA[p,j] = b_j*W[p]
                                 (split ~4:1 between ScalarE and VectorE)
  DMA: out tiles -> HBM
"""

import numpy as np

import concourse.bacc as bacc
import concourse.mybir as mybir
from concourse.tile import TileContext
from concourse.bass_utils import run_bass_kernel_spmd
from bass_rust import add_dep_helper as _add_dep

B, F, E = 256, 39, 16
EE = E * E
I = 741  # strict lower triangle of (39, 39)
P = 128
NB = B // P  # 2 batch chunks on partitions
NE = EE // P  # 2 (e1,e2) chunks on partitions after transpose
NCORES = 8
JPC = 93  # padded channels per core (8*93 = 744 >= 741)
GJ = 8  # channels per pipeline group
NLD = 6  # input load interleave chunks
EPS = 1e-5
FCE = F * E  # 624

# blob column offsets (fp32 elements; blob is (128, BLOBZ))
_Z_XJC = 0                    # (P, NB*FCE)
_Z_XIC = _Z_XJC + NB * FCE    # (P, NB*FCE)
_Z_ID = _Z_XIC + NB * FCE     # (P, P) identity
_Z_G2 = _Z_ID + P             # (F, F) gamma scattered
_Z_B2 = _Z_G2 + F             # (F, F) beta scattered
_Z_GR = _Z_B2 + F             # (F, JPC) one-hot row select
_Z_GCD = _Z_GR + JPC          # (JPC, F) one-hot col mask
_Z_W = _Z_GCD + F             # (1, EE) W flat
BLOBZ = _Z_W + EE

_ROWS, _COLS = np.tril_indices(F, k=-1)

# module-level knobs for the test harness
TRACE = False
LAST_RESULT = None

_cached_nc = None

_f32 = mybir.dt.float32
_mult = mybir.AluOpType.mult
_add = mybir.AluOpType.add


def _groups():
    # small lead-in groups keep DVE stalls short while the stats chain is
    # still running; small tail groups shrink the final drain
    g = [(0, 2), (2, 2), (4, 2), (6, 2), (8, 2), (10, 2), (12, 2), (14, 2)]
    j0 = 16
    while j0 < 88:
        g.append((j0, min(GJ, 88 - j0)))
        j0 += GJ
    g += [(88, 2), (90, 2), (92, 1)]
    return g


def _ld_chunks():
    g = []
    step = (JPC + NLD - 1) // NLD
    j0 = 0
    while j0 < JPC:
        g.append((j0, min(step, JPC - j0)))
        j0 += step
    return g


def _build_program():
    nc = bacc.Bacc("TRN2", target_bir_lowering=False, debug=False,
                   num_devices=NCORES)
    FJE = JPC * E  # 1488

    ag = nc.dram_tensor("ag", (P, NB, FJE), _f32, kind="ExternalInput")
    xg = nc.dram_tensor("xg", (P, NB, FJE), _f32, kind="ExternalInput")
    blob = nc.dram_tensor("blob", (P, BLOBZ), _f32, kind="ExternalInput")
    # out[j, p, c2, c, b128] = result[b=c*128+b128, i=j, e=c2*128+p]
    out_t = nc.dram_tensor("out", (JPC, P, NE * NB * P), _f32,
                           kind="ExternalOutput")

    Ident = mybir.ActivationFunctionType.Identity

    with TileContext(nc) as tc:
        with tc.tile_pool(name="const", bufs=1) as cpool, \
             tc.tile_pool(name="work", bufs=3) as wpool:
            ppool_cm = tc.tile_pool(name="psum_st", bufs=1, space="PSUM")
            ppool = ppool_cm.__enter__()

            # --- one packed DMA for everything the stats path needs ---
            blob_sb = cpool.tile([P, BLOBZ], _f32)
            nc.sync.dma_start(blob_sb[:, :_Z_ID], blob[:, :_Z_ID])
            nc.sync.dma_start(blob_sb[:, _Z_ID:], blob[:, _Z_ID:])

            bv = blob_sb[:]
            xjc_sb = bv[:, _Z_XJC:_Z_XJC + NB * FCE]
            xic_sb = bv[:, _Z_XIC:_Z_XIC + NB * FCE]
            id_sb = bv[:, _Z_ID:_Z_ID + P]
            g2_sb = bv[:F, _Z_G2:_Z_G2 + F]
            b2_sb = bv[:F, _Z_B2:_Z_B2 + F]
            gr_sb = bv[:F, _Z_GR:_Z_GR + JPC]
            gcd_sb = bv[:JPC, _Z_GCD:_Z_GCD + F]
            w_sb = bv[:1, _Z_W:_Z_W + EE]

            # --- big gathered loads, interleaved by channel range ---
            a_sb = cpool.tile([P, NB * FJE], _f32)
            x_sb = cpool.tile([P, NB * FJE], _f32)
            a3 = a_sb[:].rearrange("p (c j e) -> p c j e", c=NB, j=JPC)
            x3 = x_sb[:].rearrange("p (c j e) -> p c j e", c=NB, j=JPC)
            ag3 = ag[:].rearrange("p c (j e) -> p c j e", j=JPC)
            xg3 = xg[:].rearrange("p c (j e) -> p c j e", j=JPC)
            for (r0, rn) in _ld_chunks():
                nc.sync.dma_start(a3[:, :, r0:r0 + rn, :],
                                  ag3[:, :, r0:r0 + rn, :])
                nc.sync.dma_start(x3[:, :, r0:r0 + rn, :],
                                  xg3[:, :, r0:r0 + rn, :])

            # ---- stats from compact tensors ----
            sJ = cpool.tile([P, NB * F], _f32)
            sI = cpool.tile([P, NB * F], _f32)
            qJ = cpool.tile([P, NB * F], _f32)
            qI = cpool.tile([P, NB * F], _f32)
            nc.vector.tensor_reduce(
                sJ[:].rearrange("p (c f) -> p c f", c=NB),
                xjc_sb.rearrange("p (c f e) -> p c f e", c=NB, f=F),
                mybir.AxisListType.X, _add)
            nc.vector.tensor_reduce(
                sI[:].rearrange("p (c f) -> p c f", c=NB),
                xic_sb.rearrange("p (c f e) -> p c f e", c=NB, f=F),
                mybir.AxisListType.X, _add)

            # M1[r,c] = sum_b sJ[b,r]*sI[b,c]
            m1_ps = ppool.tile([F, F], _f32, tag="m1")
            for c in range(NB):
                nc.tensor.matmul(m1_ps[:], sJ[:, c * F:(c + 1) * F],
                                 sI[:, c * F:(c + 1) * F],
                                 start=(c == 0), stop=(c == NB - 1))

            sqJ = cpool.tile([P, NB * FCE], _f32)
            sqI = cpool.tile([P, NB * FCE], _f32)
            nc.scalar.square(sqJ[:], xjc_sb)
            nc.scalar.square(sqI[:], xic_sb)
            nc.vector.tensor_reduce(
                qJ[:].rearrange("p (c f) -> p c f", c=NB),
                sqJ[:].rearrange("p (c f e) -> p c f e", c=NB, f=F),
                mybir.AxisListType.X, _add)
            nc.vector.tensor_reduce(
                qI[:].rearrange("p (c f) -> p c f", c=NB),
                sqI[:].rearrange("p (c f e) -> p c f e", c=NB, f=F),
                mybir.AxisListType.X, _add)
            m2_ps = ppool.tile([F, F], _f32, tag="m2")
            for c in range(NB):
                nc.tensor.matmul(m2_ps[:], qJ[:, c * F:(c + 1) * F],
                                 qI[:, c * F:(c + 1) * F],
                                 start=(c == 0), stop=(c == NB - 1))

            inv_n = 1.0 / float(B * EE)
            mean2 = cpool.tile([F, F], _f32)
            msq2 = cpool.tile([F, F], _f32)
            nc.vector.tensor_scalar_mul(mean2[:], m1_ps[:], inv_n)
            nc.vector.tensor_scalar_mul(msq2[:], m2_ps[:], inv_n)

            m2sq = cpool.tile([F, F], _f32)
            varp = cpool.tile([F, F], _f32)
            nc.vector.tensor_mul(m2sq[:], mean2[:], mean2[:])
            nc.vector.tensor_sub(varp[:], msq2[:], m2sq[:])
            nc.vector.tensor_scalar_add(varp[:], varp[:], EPS)

            # rstd = rsqrt(var+eps): reciprocal -> sqrt -> one Newton step
            inv = cpool.tile([F, F], _f32)
            r0t = cpool.tile([F, F], _f32)
            nc.vector.reciprocal(inv[:], varp[:])
            nc.scalar.sqrt(r0t[:], inv[:])
            r0sq = cpool.tile([F, F], _f32)
            ut = cpool.tile([F, F], _f32)
            rstd = cpool.tile([F, F], _f32)
            nc.vector.tensor_mul(r0sq[:], r0t[:], r0t[:])
            nc.vector.tensor_mul(ut[:], varp[:], r0sq[:])
            nc.vector.tensor_scalar(ut[:], ut[:], -0.5, 1.5, _mult, _add)
            nc.vector.tensor_mul(rstd[:], r0t[:], ut[:])

            sc2 = cpool.tile([F, F], _f32)
            bi2 = cpool.tile([F, F], _f32)
            tmp2 = cpool.tile([F, F], _f32)
            nc.vector.tensor_mul(sc2[:], g2_sb, rstd[:])
            nc.vector.tensor_mul(tmp2[:], mean2[:], sc2[:])
            nc.vector.tensor_sub(bi2[:], b2_sb, tmp2[:])

            # gather per-channel values: v[j] = V[rows[j], cols[j]]
            # rows via one-hot matmul, cols via masked free-reduce,
            # then transpose the (93,1) column to a (1,93) row via PE.
            def gather_row(v2d, name):
                gps = ppool.tile([JPC, F], _f32, tag=f"misc_{name}")
                nc.tensor.matmul(gps[:], gr_sb, v2d[:])
                o1 = cpool.tile([JPC, F], _f32, tag=f"o1_{name}")
                nc.vector.tensor_copy(o1[:], gps[:])
                junk = cpool.tile([JPC, F], _f32, tag=f"junk_{name}")
                vcol = cpool.tile([JPC, 1], _f32, tag=f"vcol_{name}")
                nc.vector.tensor_mul(junk[:], o1[:], gcd_sb)
                nc.vector.tensor_reduce(vcol[:], junk[:],
                                        mybir.AxisListType.X, _add)
                vrow_ps = ppool.tile([1, JPC], _f32, tag=f"misc_{name}")
                nc.tensor.matmul(vrow_ps[:], vcol[:], id_sb[:JPC, :JPC])
                vrow = cpool.tile([1, JPC], _f32, tag=f"vr_{name}")
                nc.vector.tensor_copy(vrow[:], vrow_ps[:])
                return vrow

            scv = gather_row(sc2, "s")
            biv = gather_row(bi2, "b")

            # VM[p, c2, j] = W[c2*128+p] * s_j ; VA likewise with b_j
            # both packed into one PSUM bank, evacuated with one copy
            vv = cpool.tile([P, 2 * NE * JPC], _f32)
            vv_ps = ppool.tile([P, 2 * NE * JPC], _f32, tag="vvps")
            for c2 in range(NE):
                wt = w_sb[:, c2 * P:(c2 + 1) * P]
                nc.tensor.matmul(vv_ps[:, c2 * JPC:(c2 + 1) * JPC], wt, scv[:],
                                 start=True, stop=True)
                nc.tensor.matmul(
                    vv_ps[:, (NE + c2) * JPC:(NE + c2 + 1) * JPC], wt, biv[:],
                    start=True, stop=True)
            vv_inst = nc.vector.tensor_copy(vv[:], vv_ps[:])
            vm = vv[:, :NE * JPC]
            va = vv[:, NE * JPC:]

            # release the stats PSUM banks; the transpose pool gets all 8
            ppool_cm.__exit__(None, None, None)
            trpool_cm = tc.tile_pool(name="psum_tr", bufs=8, space="PSUM")
            trpool = trpool_cm.__enter__()

            # ---- main pipeline over channel groups of GJ ----
            n_aff = 0
            for (g0, gn) in _groups():
                t1 = wpool.tile([P, NB * GJ * EE], _f32, tag="t1")
                t1v = t1[:].rearrange("p (c j f) -> p c j f", c=NB, j=GJ)

                for c in range(NB):
                    a_ap = a3[:, c, g0:g0 + gn, :].unsqueeze(3) \
                        .broadcast_to((P, gn, E, E))
                    x_ap = x3[:, c, g0:g0 + gn, :].unsqueeze(2) \
                        .broadcast_to((P, gn, E, E))
                    o_ap = t1v[:, c, :gn, :].rearrange(
                        "p j (e1 e2) -> p j e1 e2", e1=E)
                    t1_inst = nc.vector.tensor_tensor(o_ap, a_ap, x_ap, _mult)
                    if gn > 2:
                        # keep the DVE queue clear for the tiny stats-chain
                        # ops: order the bulk outer-product ops after the
                        # chain's last DVE op (same engine -> pure ordering)
                        _add_dep(t1_inst.ins, vv_inst.ins, sync=False,
                                 reason="stats chain before bulk outer products")

                og = wpool.tile([P, GJ * NE * NB * P], _f32, tag="og")
                og4 = og[:].rearrange("p (j c2 f) -> p j c2 f", j=GJ, c2=NE)

                for jj in range(gn):
                    j = g0 + jj
                    # one PSUM bank per channel: 4 transposed 128x128 tiles
                    tr = trpool.tile([P, NE * NB * P], _f32, tag="tr")
                    tr4 = tr[:].rearrange("p (c2 c b) -> p c2 c b", c2=NE, c=NB)
                    for c2 in range(NE):
                        for c in range(NB):
                            nc.tensor.transpose(
                                tr4[:, c2, c, :],
                                t1v[:, c, jj, c2 * P:(c2 + 1) * P],
                                id_sb)
                    for c2 in range(NE):
                        n_aff += 1
                        if n_aff % 5 == 0:
                            # ~1/5 of the affine ops on VectorE
                            nc.vector.tensor_scalar(
                                og4[:, jj, c2, :], tr4[:, c2, :, :],
                                vm[:, c2 * JPC + j:c2 * JPC + j + 1],
                                va[:, c2 * JPC + j:c2 * JPC + j + 1],
                                _mult, _add)
                        else:
                            nc.scalar.activation(
                                og4[:, jj, c2, :], tr4[:, c2, :, :], Ident,
                                bias=va[:, c2 * JPC + j:c2 * JPC + j + 1],
                                scale=vm[:, c2 * JPC + j:c2 * JPC + j + 1])

                nc.sync.dma_start(
                    out_t[g0:g0 + gn, :, :].rearrange("j p f -> p j f"),
                    og4[:, :gn, :, :].rearrange("p j c2 f -> p j (c2 f)"))

            trpool_cm.__exit__(None, None, None)

    nc.compile()
    return nc


def _shard_inputs(xi, xj, W, gamma, beta):
    """Host-side gather: per-core per-channel A (= xj rows) / X (= xi cols),
    plus a packed blob of compact tensors and static one-hot index maps."""
    FJE = JPC * E
    # shared pieces
    xjc = xj.reshape(NB, P, FCE).transpose(1, 0, 2).reshape(P, NB * FCE)
    xic = xi.reshape(NB, P, FCE).transpose(1, 0, 2).reshape(P, NB * FCE)
    g2d = np.ones((F, F), dtype=np.float32)
    b2d = np.zeros((F, F), dtype=np.float32)
    g2d[_ROWS, _COLS] = gamma
    b2d[_ROWS, _COLS] = beta

    in_maps = []
    for k in range(NCORES):
        j0 = k * JPC
        j1 = min(j0 + JPC, I)
        nj = j1 - j0
        rows = np.ones(JPC, dtype=np.int64)  # pad -> (1, 0)
        cols = np.zeros(JPC, dtype=np.int64)
        rows[:nj] = _ROWS[j0:j1]
        cols[:nj] = _COLS[j0:j1]

        a_k = np.zeros((P, NB, FJE), dtype=np.float32)
        x_k = np.zeros((P, NB, FJE), dtype=np.float32)
        a_full = xj[:, rows[:nj], :].reshape(NB, P, nj * E)
        x_full = xi[:, cols[:nj], :].reshape(NB, P, nj * E)
        a_k[:, :, :nj * E] = a_full.transpose(1, 0, 2)
        x_k[:, :, :nj * E] = x_full.transpose(1, 0, 2)

        bl = np.zeros((P, BLOBZ), dtype=np.float32)
        bl[:, _Z_XJC:_Z_XJC + NB * FCE] = xjc
        bl[:, _Z_XIC:_Z_XIC + NB * FCE] = xic
        bl[:, _Z_ID:_Z_ID + P] = np.eye(P, dtype=np.float32)
        bl[:F, _Z_G2:_Z_G2 + F] = g2d
        bl[:F, _Z_B2:_Z_B2 + F] = b2d
        bl[rows, _Z_GR + np.arange(JPC)] = 1.0
        bl[np.arange(JPC), _Z_GCD + cols] = 1.0
        bl[0, _Z_W:_Z_W + EE] = W.reshape(EE)

        in_maps.append({"ag": a_k, "xg": x_k, "blob": bl})
    return in_maps


def kernel(xi, xj, W, gamma, beta):
    global _cached_nc, LAST_RESULT
    xi = np.ascontiguousarray(np.asarray(xi), dtype=np.float32)
    xj = np.ascontiguousarray(np.asarray(xj), dtype=np.float32)
    W = np.asarray(W, dtype=np.float32)
    gamma = np.asarray(gamma, dtype=np.float32)
    beta = np.asarray(beta, dtype=np.float32)

    if _cached_nc is None:
        _cached_nc = _build_program()
    nc = _cached_nc

    in_maps = _shard_inputs(xi, xj, W, gamma, beta)
    res = run_bass_kernel_spmd(nc, in_maps, core_ids=list(range(NCORES)),
                               trace=TRACE)
    LAST_RESULT = res

    full = np.empty((B, I, EE), dtype=np.float32)
    for k in range(NCORES):
        j0 = k * JPC
        j1 = min(j0 + JPC, I)
        nj = j1 - j0
        r = res.results[k]["out"].reshape(JPC, P, NE, NB, P)
        # r[j, p, c2, c, b128] = out[b=c*128+b128, i=j0+j, e=c2*128+p]
        full[:, j0:j1, :] = (
            r[:nj].transpose(3, 4, 0, 2, 1).reshape(B, nj, EE))
    return full.reshape(B, 1, I, E, E)


# revision 22
# speedup vs baseline: 1.0635x; 1.0221x over previous
"""Trainium2 Bass kernel for nn_CrossLayer (B=256, F=39, E=16, I=741, C=1).

out[b, 0, i, e1, e2] = BN(cross)[b,i,e1,e2] * W[0,e1,e2]
  cross[b,i,e1,e2] = xj[b, rows[i], e1] * xi[b, cols[i], e2]
  BN over channel i with training-mode batch stats across (b, e1, e2).

Sharding: channels I=741 split across 8 cores (93 per core, zero-padded).
Each core sees the full batch for its channels, so BN stats are fully
local (no collectives).

BN stats are computed analytically from the COMPACT per-feature sums:
  mean[r,c]  = (1/(B*E^2)) * sum_b sJ[b,r]*sI[b,c]   (39x39 PE matmul)
  E[x^2][r,c]= (1/(B*E^2)) * sum_b qJ[b,r]*qI[b,c]
then per-channel scale/bias are gathered from the 39x39 maps with a
one-hot PE matmul (rows) + masked free-reduce (cols).  The stats
operands ride in a single packed "blob" DMA (~1.6 MB) so the chain can
finish while the big gathered operands are still streaming in.

Per-core device pipeline:
  DVE: t1 = A (x) X              outer product, b on partitions
                                 (stride-0 broadcast APs)
  PE : transpose 128x128 tiles   t1 -> PSUM, (e1,e2) now on partitions
  ACT/DVE: out = t1_T*VM + VA    per-partition scale/bias fuses the BN
                                 affine AND the W multiply in one op:
                                 VM[p,j] = s_j*W[p], VA[p,j] = b_j*W[p]
                                 (split ~4:1 between ScalarE and VectorE)
  DMA: out tiles -> HBM
"""

import numpy as np

import concourse.bacc as bacc
import concourse.mybir as mybir
from concourse.tile import TileContext
from concourse.bass_utils import run_bass_kernel_spmd
from bass_rust import add_dep_helper as _add_dep

B, F, E = 256, 39, 16
EE = E * E
I = 741  # strict lower triangle of (39, 39)
P = 128
NB = B // P  # 2 batch chunks on partitions
NE = EE // P  # 2 (e1,e2) chunks on partitions after transpose
NCORES = 8
JPC = 93  # padded channels per core (8*93 = 744 >= 741)
GJ = 8  # channels per pipeline group
NLD = 4  # input load interleave chunks
EPS = 1e-5
FCE = F * E  # 624

# blob column offsets (fp32 elements; blob is (128, BLOBZ))
_Z_XJC = 0                    # (P, NB*FCE)
_Z_XIC = _Z_XJC + NB * FCE    # (P, NB*FCE)
_Z_ID = _Z_XIC + NB * FCE     # (P, P) identity
_Z_G2 = _Z_ID + P             # (F, F) gamma scattered
_Z_B2 = _Z_G2 + F             # (F, F) beta scattered
_Z_GR = _Z_B2 + F             # (F, JPC) one-hot row select
_Z_GCD = _Z_GR + JPC          # (JPC, F) one-hot col mask
_Z_W = _Z_GCD + F             # (1, EE) W flat
BLOBZ = _Z_W + EE

_ROWS, _COLS = np.tril_indices(F, k=-1)

# module-level knobs for the test harness
TRACE = False
LAST_RESULT = None

_cached_nc = None

_f32 = mybir.dt.float32
_mult = mybir.AluOpType.mult
_add = mybir.AluOpType.add


def _groups():
    # small lead-in groups keep DVE stalls short while the stats chain is
    # still running; small tail groups shrink the final drain
    g = [(0, 2), (2, 2), (4, 2), (6, 2), (8, 2), (10, 2), (12, 2), (14, 2)]
    j0 = 16
    while j0 < 88:
        g.append((j0, min(GJ, 88 - j0)))
        j0 += GJ
    g += [(88, 2), (90, 2), (92, 1)]
    return g


def _ld_chunks():
    g = []
    step = (JPC + NLD - 1) // NLD
    j0 = 0
    while j0 < JPC:
        g.append((j0, min(step, JPC - j0)))
        j0 += step
    return g


def _build_program():
    nc = bacc.Bacc("TRN2", target_bir_lowering=False, debug=False,
                   num_devices=NCORES)
    FJE = JPC * E  # 1488

    ag = nc.dram_tensor("ag", (P, NB, FJE), _f32, kind="ExternalInput")
    xg = nc.dram_tensor("xg", (P, NB, FJE), _f32, kind="ExternalInput")
    blob = nc.dram_tensor("blob", (P, BLOBZ), _f32, kind="ExternalInput")
    # out[j, p, c2, c, b128] = result[b=c*128+b128, i=j, e=c2*128+p]
    out_t = nc.dram_tensor("out", (JPC, P, NE * NB * P), _f32,
                           kind="ExternalOutput")

    Ident = mybir.ActivationFunctionType.Identity

    with TileContext(nc) as tc:
        with tc.tile_pool(name="const", bufs=1) as cpool, \
             tc.tile_pool(name="work", bufs=3) as wpool:
            ppool_cm = tc.tile_pool(name="psum_st", bufs=1, space="PSUM")
            ppool = ppool_cm.__enter__()

            # --- one packed DMA for everything the stats path needs ---
            blob_sb = cpool.tile([P, BLOBZ], _f32)
            nc.sync.dma_start(blob_sb[:, :_Z_ID], blob[:, :_Z_ID])
            nc.sync.dma_start(blob_sb[:, _Z_ID:], blob[:, _Z_ID:])

            bv = blob_sb[:]
            xjc_sb = bv[:, _Z_XJC:_Z_XJC + NB * FCE]
            xic_sb = bv[:, _Z_XIC:_Z_XIC + NB * FCE]
            id_sb = bv[:, _Z_ID:_Z_ID + P]
            g2_sb = bv[:F, _Z_G2:_Z_G2 + F]
            b2_sb = bv[:F, _Z_B2:_Z_B2 + F]
            gr_sb = bv[:F, _Z_GR:_Z_GR + JPC]
            gcd_sb = bv[:JPC, _Z_GCD:_Z_GCD + F]
            w_sb = bv[:1, _Z_W:_Z_W + EE]

            # --- big gathered loads, interleaved by channel range ---
            a_sb = cpool.tile([P, NB * FJE], _f32)
            x_sb = cpool.tile([P, NB * FJE], _f32)
            a3 = a_sb[:].rearrange("p (c j e) -> p c j e", c=NB, j=JPC)
            x3 = x_sb[:].rearrange("p (c j e) -> p c j e", c=NB, j=JPC)
            ag3 = ag[:].rearrange("p c (j e) -> p c j e", j=JPC)
            xg3 = xg[:].rearrange("p c (j e) -> p c j e", j=JPC)
            for (r0, rn) in _ld_chunks():
                nc.sync.dma_start(a3[:, :, r0:r0 + rn, :],
                                  ag3[:, :, r0:r0 + rn, :])
                nc.sync.dma_start(x3[:, :, r0:r0 + rn, :],
                                  xg3[:, :, r0:r0 + rn, :])

            # ---- stats from compact tensors ----
            sJ = cpool.tile([P, NB * F], _f32)
            sI = cpool.tile([P, NB * F], _f32)
            qJ = cpool.tile([P, NB * F], _f32)
            qI = cpool.tile([P, NB * F], _f32)
            nc.vector.tensor_reduce(
                sJ[:].rearrange("p (c f) -> p c f", c=NB),
                xjc_sb.rearrange("p (c f e) -> p c f e", c=NB, f=F),
                mybir.AxisListType.X, _add)
            nc.vector.tensor_reduce(
                sI[:].rearrange("p (c f) -> p c f", c=NB),
                xic_sb.rearrange("p (c f e) -> p c f e", c=NB, f=F),
                mybir.AxisListType.X, _add)

            # M1[r,c] = sum_b sJ[b,r]*sI[b,c]
            m1_ps = ppool.tile([F, F], _f32, tag="m1")
            for c in range(NB):
                nc.tensor.matmul(m1_ps[:], sJ[:, c * F:(c + 1) * F],
                                 sI[:, c * F:(c + 1) * F],
                                 start=(c == 0), stop=(c == NB - 1))

            sqJ = cpool.tile([P, NB * FCE], _f32)
            sqI = cpool.tile([P, NB * FCE], _f32)
            nc.scalar.square(sqJ[:], xjc_sb)
            nc.scalar.square(sqI[:], xic_sb)
            nc.vector.tensor_reduce(
                qJ[:].rearrange("p (c f) -> p c f", c=NB),
                sqJ[:].rearrange("p (c f e) -> p c f e", c=NB, f=F),
                mybir.AxisListType.X, _add)
            nc.vector.tensor_reduce(
                qI[:].rearrange("p (c f) -> p c f", c=NB),
                sqI[:].rearrange("p (c f e) -> p c f e", c=NB, f=F),
                mybir.AxisListType.X, _add)
            m2_ps = ppool.tile([F, F], _f32, tag="m2")
            for c in range(NB):
                nc.tensor.matmul(m2_ps[:], qJ[:, c * F:(c + 1) * F],
                                 qI[:, c * F:(c + 1) * F],
                                 start=(c == 0), stop=(c == NB - 1))

            inv_n = 1.0 / float(B * EE)
            mean2 = cpool.tile([F, F], _f32)
            msq2 = cpool.tile([F, F], _f32)
            nc.vector.tensor_scalar_mul(mean2[:], m1_ps[:], inv_n)
            nc.vector.tensor_scalar_mul(msq2[:], m2_ps[:], inv_n)

            m2sq = cpool.tile([F, F], _f32)
            varp = cpool.tile([F, F], _f32)
            nc.vector.tensor_mul(m2sq[:], mean2[:], mean2[:])
            nc.vector.tensor_sub(varp[:], msq2[:], m2sq[:])
            nc.vector.tensor_scalar_add(varp[:], varp[:], EPS)

            # rstd = rsqrt(var+eps): reciprocal -> sqrt -> one Newton step
            inv = cpool.tile([F, F], _f32)
            r0t = cpool.tile([F, F], _f32)
            nc.vector.reciprocal(inv[:], varp[:])
            nc.scalar.sqrt(r0t[:], inv[:])
            r0sq = cpool.tile([F, F], _f32)
            ut = cpool.tile([F, F], _f32)
            rstd = cpool.tile([F, F], _f32)
            nc.vector.tensor_mul(r0sq[:], r0t[:], r0t[:])
            nc.vector.tensor_mul(ut[:], varp[:], r0sq[:])
            nc.vector.tensor_scalar(ut[:], ut[:], -0.5, 1.5, _mult, _add)
            nc.vector.tensor_mul(rstd[:], r0t[:], ut[:])

            sc2 = cpool.tile([F, F], _f32)
            bi2 = cpool.tile([F, F], _f32)
            tmp2 = cpool.tile([F, F], _f32)
            nc.vector.tensor_mul(sc2[:], g2_sb, rstd[:])
            nc.vector.tensor_mul(tmp2[:], mean2[:], sc2[:])
            nc.vector.tensor_sub(bi2[:], b2_sb, tmp2[:])

            # gather per-channel values: v[j] = V[rows[j], cols[j]]
            # rows via one-hot matmul, cols via masked free-reduce,
            # then transpose the (93,1) column to a (1,93) row via PE.
            def gather_row(v2d, name):
                gps = ppool.tile([JPC, F], _f32, tag=f"misc_{name}")
                nc.tensor.matmul(gps[:], gr_sb, v2d[:])
                o1 = cpool.tile([JPC, F], _f32, tag=f"o1_{name}")
                nc.vector.tensor_copy(o1[:], gps[:])
                junk = cpool.tile([JPC, F], _f32, tag=f"junk_{name}")
                vcol = cpool.tile([JPC, 1], _f32, tag=f"vcol_{name}")
                nc.vector.tensor_mul(junk[:], o1[:], gcd_sb)
                nc.vector.tensor_reduce(vcol[:], junk[:],
                                        mybir.AxisListType.X, _add)
                vrow_ps = ppool.tile([1, JPC], _f32, tag=f"misc_{name}")
                nc.tensor.matmul(vrow_ps[:], vcol[:], id_sb[:JPC, :JPC])
                vrow = cpool.tile([1, JPC], _f32, tag=f"vr_{name}")
                nc.vector.tensor_copy(vrow[:], vrow_ps[:])
                return vrow

            scv = gather_row(sc2, "s")
            biv = gather_row(bi2, "b")

            # VM[p, c2, j] = W[c2*128+p] * s_j ; VA likewise with b_j
            # both packed into one PSUM bank, evacuated with one copy
            vv = cpool.tile([P, 2 * NE * JPC], _f32)
            vv_ps = ppool.tile([P, 2 * NE * JPC], _f32, tag="vvps")
            for c2 in range(NE):
                wt = w_sb[:, c2 * P:(c2 + 1) * P]
                nc.tensor.matmul(vv_ps[:, c2 * JPC:(c2 + 1) * JPC], wt, scv[:],
                                 start=True, stop=True)
                nc.tensor.matmul(
                    vv_ps[:, (NE + c2) * JPC:(NE + c2 + 1) * JPC], wt, biv[:],
                    start=True, stop=True)
            vv_inst = nc.vector.tensor_copy(vv[:], vv_ps[:])
            vm = vv[:, :NE * JPC]
            va = vv[:, NE * JPC:]

            # release the stats PSUM banks; the transpose pool gets all 8
            ppool_cm.__exit__(None, None, None)
            trpool_cm = tc.tile_pool(name="psum_tr", bufs=8, space="PSUM")
            trpool = trpool_cm.__enter__()

            # ---- main pipeline over channel groups of GJ ----
            n_aff = 0
            for (g0, gn) in _groups():
                t1 = wpool.tile([P, NB * GJ * EE], _f32, tag="t1")
                t1v = t1[:].rearrange("p (c j f) -> p c j f", c=NB, j=GJ)

                for c in range(NB):
                    a_ap = a3[:, c, g0:g0 + gn, :].unsqueeze(3) \
                        .broadcast_to((P, gn, E, E))
                    x_ap = x3[:, c, g0:g0 + gn, :].unsqueeze(2) \
                        .broadcast_to((P, gn, E, E))
                    o_ap = t1v[:, c, :gn, :].rearrange(
                        "p j (e1 e2) -> p j e1 e2", e1=E)
                    t1_inst = nc.vector.tensor_tensor(o_ap, a_ap, x_ap, _mult)
                    if gn > 2:
                        # keep the DVE queue clear for the tiny stats-chain
                        # ops: order the bulk outer-product ops after the
                        # chain's last DVE op (same engine -> pure ordering)
                        _add_dep(t1_inst.ins, vv_inst.ins, sync=False,
                                 reason="stats chain before bulk outer products")

                og = wpool.tile([P, GJ * NE * NB * P], _f32, tag="og")
                og4 = og[:].rearrange("p (j c2 f) -> p j c2 f", j=GJ, c2=NE)

                for jj in range(gn):
                    j = g0 + jj
                    # one PSUM bank per channel: 4 transposed 128x128 tiles
                    tr = trpool.tile([P, NE * NB * P], _f32, tag="tr")
                    tr4 = tr[:].rearrange("p (c2 c b) -> p c2 c b", c2=NE, c=NB)
                    for c2 in range(NE):
                        for c in range(NB):
                            nc.tensor.transpose(
                                tr4[:, c2, c, :],
                                t1v[:, c, jj, c2 * P:(c2 + 1) * P],
                                id_sb)
                    for c2 in range(NE):
                        n_aff += 1
                        if n_aff % 6 == 0:
                            # ~1/5 of the affine ops on VectorE
                            nc.vector.tensor_scalar(
                                og4[:, jj, c2, :], tr4[:, c2, :, :],
                                vm[:, c2 * JPC + j:c2 * JPC + j + 1],
                                va[:, c2 * JPC + j:c2 * JPC + j + 1],
                                _mult, _add)
                        else:
                            nc.scalar.activation(
                                og4[:, jj, c2, :], tr4[:, c2, :, :], Ident,
                                bias=va[:, c2 * JPC + j:c2 * JPC + j + 1],
                                scale=vm[:, c2 * JPC + j:c2 * JPC + j + 1])

                nc.sync.dma_start(
                    out_t[g0:g0 + gn, :, :].rearrange("j p f -> p j f"),
                    og4[:, :gn, :, :].rearrange("p j c2 f -> p j (c2 f)"))

            trpool_cm.__exit__(None, None, None)

    nc.compile()
    return nc


def _shard_inputs(xi, xj, W, gamma, beta):
    """Host-side gather: per-core per-channel A (= xj rows) / X (= xi cols),
    plus a packed blob of compact tensors and static one-hot index maps."""
    FJE = JPC * E
    # shared pieces
    xjc = xj.reshape(NB, P, FCE).transpose(1, 0, 2).reshape(P, NB * FCE)
    xic = xi.reshape(NB, P, FCE).transpose(1, 0, 2).reshape(P, NB * FCE)
    g2d = np.ones((F, F), dtype=np.float32)
    b2d = np.zeros((F, F), dtype=np.float32)
    g2d[_ROWS, _COLS] = gamma
    b2d[_ROWS, _COLS] = beta

    in_maps = []
    for k in range(NCORES):
        j0 = k * JPC
        j1 = min(j0 + JPC, I)
        nj = j1 - j0
        rows = np.ones(JPC, dtype=np.int64)  # pad -> (1, 0)
        cols = np.zeros(JPC, dtype=np.int64)
        rows[:nj] = _ROWS[j0:j1]
        cols[:nj] = _COLS[j0:j1]

        a_k = np.zeros((P, NB, FJE), dtype=np.float32)
        x_k = np.zeros((P, NB, FJE), dtype=np.float32)
        a_full = xj[:, rows[:nj], :].reshape(NB, P, nj * E)
        x_full = xi[:, cols[:nj], :].reshape(NB, P, nj * E)
        a_k[:, :, :nj * E] = a_full.transpose(1, 0, 2)
        x_k[:, :, :nj * E] = x_full.transpose(1, 0, 2)

        bl = np.zeros((P, BLOBZ), dtype=np.float32)
        bl[:, _Z_XJC:_Z_XJC + NB * FCE] = xjc
        bl[:, _Z_XIC:_Z_XIC + NB * FCE] = xic
        bl[:, _Z_ID:_Z_ID + P] = np.eye(P, dtype=np.float32)
        bl[:F, _Z_G2:_Z_G2 + F] = g2d
        bl[:F, _Z_B2:_Z_B2 + F] = b2d
        bl[rows, _Z_GR + np.arange(JPC)] = 1.0
        bl[np.arange(JPC), _Z_GCD + cols] = 1.0
        bl[0, _Z_W:_Z_W + EE] = W.reshape(EE)

        in_maps.append({"ag": a_k, "xg": x_k, "blob": bl})
    return in_maps


def kernel(xi, xj, W, gamma, beta):
    global _cached_nc, LAST_RESULT
    xi = np.ascontiguousarray(np.asarray(xi), dtype=np.float32)
    xj = np.ascontiguousarray(np.asarray(xj), dtype=np.float32)
    W = np.asarray(W, dtype=np.float32)
    gamma = np.asarray(gamma, dtype=np.float32)
    beta = np.asarray(beta, dtype=np.float32)

    if _cached_nc is None:
        _cached_nc = _build_program()
    nc = _cached_nc

    in_maps = _shard_inputs(xi, xj, W, gamma, beta)
    res = run_bass_kernel_spmd(nc, in_maps, core_ids=list(range(NCORES)),
                               trace=TRACE)
    LAST_RESULT = res

    full = np.empty((B, I, EE), dtype=np.float32)
    for k in range(NCORES):
        j0 = k * JPC
        j1 = min(j0 + JPC, I)
        nj = j1 - j0
        r = res.results[k]["out"].reshape(JPC, P, NE, NB, P)
        # r[j, p, c2, c, b128] = out[b=c*128+b128, i=j0+j, e=c2*128+p]
        full[:, j0:j1, :] = (
            r[:nj].transpose(3, 4, 0, 2, 1).reshape(B, nj, EE))
    return full.reshape(B, 1, I, E, E)
